# revision 8
# baseline (speedup 1.0000x reference)
"""Trainium2 Bass kernel for nn_CycleMultiBlock (self-contained).

Model: a = emb[x]; L x [a (+)= ((scan(LN(a)@Wx) * (LN(a)@Wv)) @ Wh)]; LN; MLP head.
The linear RNN h_t = R(theta) h_{t-1} + u_t (2x2 rotation blocks, theta constant
over time) is factored as h_t = e^{i t th} * cumsum_k(e^{-i k th} u_k), computed
per 128-token tile with a triangular-ones matmul; the cross-tile carry is
injected into PSUM with a K=1 ones-row matmul.

Sharding: data-parallel over batch; 2 sequences per core, 8 cores.
Activations token-major [128 tok, D]; lhsT tiles via PE transposes.
Matmuls in float32r (full-rate fp32, ~1e-4 input rounding).
"""
import sys

sys.path.insert(0, "/opt/trn_rl_repo")

import numpy as np

import concourse.bacc as bacc
import concourse.mybir as mybir
import concourse.tile as tile

B, S, M, D, L = 16, 2048, 128, 1024, 4
P = 128
NCORES = 8
BPC = B // NCORES          # sequences per core
TILES = S // P             # 128-token tiles per sequence
EPS = 1e-5

f32 = mybir.dt.float32
f32r = mybir.dt.float32r


def _host_tables(theta_l, d):
    th = theta_l.astype(np.float64)  # [d//2]
    k = np.arange(P, dtype=np.float64)[:, None]
    ang = k * th[None, :]
    rot = np.empty((P, d), np.float32)
    rot[:, 0::2] = np.cos(ang).astype(np.float32)
    rot[:, 1::2] = np.sin(ang).astype(np.float32)
    angp = P * th
    phi = np.empty((1, d), np.float32)
    phi[0, 0::2] = np.cos(angp).astype(np.float32)
    phi[0, 1::2] = np.sin(angp).astype(np.float32)
    return rot, phi


def build(nc, n_layers=L, n_seq=BPC, n_tiles=TILES, d=D):
    kt = d // P
    nb = d // 2
    nh = max(1, d // 512)              # 512-wide psum halves
    HW = min(512, d)
    oh = nc.dram_tensor("oh", (n_seq, n_tiles, P, P), f32r, kind="ExternalInput")
    emb = nc.dram_tensor("emb", (P, d), f32r, kind="ExternalInput")
    WX = nc.dram_tensor("WX", (n_layers, kt, P, d), f32r, kind="ExternalInput")
    WV = nc.dram_tensor("WV", (n_layers, kt, P, d), f32r, kind="ExternalInput")
    WH = nc.dram_tensor("WH", (n_layers, kt, P, d), f32r, kind="ExternalInput")
    LB = nc.dram_tensor("LB", (n_layers, 3, d), f32r, kind="ExternalInput")
    ROT = nc.dram_tensor("ROT", (n_layers, P, d), f32, kind="ExternalInput")
    PHI = nc.dram_tensor("PHI", (n_layers, 1, d), f32, kind="ExternalInput")
    TRIU = nc.dram_tensor("TRIU", (P, P), f32r, kind="ExternalInput")
    ONES1 = nc.dram_tensor("ONES1", (1, P), f32r, kind="ExternalInput")
    IDENT = nc.dram_tensor("IDENT", (P, P), f32r, kind="ExternalInput")
    WO1 = nc.dram_tensor("WO1", (kt, P, d), f32r, kind="ExternalInput")
    BO1 = nc.dram_tensor("BO1", (1, d), f32r, kind="ExternalInput")
    WO2 = nc.dram_tensor("WO2", (kt, P, M), f32r, kind="ExternalInput")
    BO2 = nc.dram_tensor("BO2", (1, M), f32r, kind="ExternalInput")
    a_dram = nc.dram_tensor("a_scratch", (n_seq, n_tiles, P, d), f32)
    OUT = nc.dram_tensor("out", (n_seq, n_tiles, P, M), f32, kind="ExternalOutput")

    sub, mult, add = (
        mybir.AluOpType.subtract,
        mybir.AluOpType.mult,
        mybir.AluOpType.add,
    )

    with tile.TileContext(nc) as tc:
        with (
            tc.tile_pool(name="consts", bufs=1) as consts,
            tc.tile_pool(name="apool", bufs=2) as apool,
            tc.tile_pool(name="stats", bufs=4) as stats,
            tc.tile_pool(name="uvps", bufs=2, space="PSUM") as uvps,
            tc.tile_pool(name="pps", bufs=1, space="PSUM") as pps,
            tc.tile_pool(name="trps", bufs=1, space="PSUM") as trps,
            tc.tile_pool(name="cps", bufs=1, space="PSUM") as cps,
        ):
            triu_sb = consts.tile([P, P], f32r)
            nc.sync.dma_start(out=triu_sb[:], in_=TRIU.ap())
            ones_sb = consts.tile([1, P], f32r)
            nc.sync.dma_start(out=ones_sb[:], in_=ONES1.ap())
            ident_sb = consts.tile([P, P], f32r)
            nc.sync.dma_start(out=ident_sb[:], in_=IDENT.ap())
            emb_sb = consts.tile([P, d], f32r)
            nc.sync.dma_start(out=emb_sb[:], in_=emb.ap())
            eps_sb = consts.tile([P, 1], f32)
            nc.vector.memset(eps_sb[:], EPS)

            def ln_xhat(pool, a_t, tag):
                st = stats.tile([P, 2, nc.vector.BN_STATS_DIM], f32, tag=f"st{tag}")
                half = d // 2
                for g in range(2):
                    nc.vector.bn_stats(
                        out=st[:, g, :], in_=a_t[:, g * half : (g + 1) * half]
                    )
                mv = stats.tile([P, nc.vector.BN_AGGR_DIM], f32, tag=f"mv{tag}")
                nc.vector.bn_aggr(out=mv[:], in_=st[:])
                rs = stats.tile([P, 1], f32, tag=f"rs{tag}")
                nc.scalar.activation(
                    out=rs[:], in_=mv[:, 1:2],
                    func=mybir.ActivationFunctionType.Sqrt,
                    bias=eps_sb[:], scale=1.0,
                )
                nc.vector.reciprocal(out=rs[:], in_=rs[:])
                xh = pool.tile([P, d], f32r, tag=f"xh{tag}", bufs=1)
                nc.vector.tensor_scalar(
                    out=xh[:], in0=a_t[:], scalar1=mv[:, 0:1], scalar2=rs[:],
                    op0=sub, op1=mult,
                )
                return xh

            def transpose_in(pool, x_t, tag):
                xT = pool.tile([P, kt, P], f32r, tag=f"xT{tag}", bufs=2)
                for k in range(kt):
                    tp = trps.tile([P, P], x_t.dtype, tag="trp")
                    nc.tensor.transpose(tp[:], x_t[:, k * P : (k + 1) * P], ident_sb[:])
                    nc.any.tensor_copy(out=xT[:, k, :], in_=tp[:])
                return xT

            def mm_full(xT, w_sb, bias_row):
                """psum [P, d] = xT.T @ W (+ bias row)."""
                ps = uvps.tile([P, d], f32, tag="uv")
                for n in range(nh):
                    nsl = slice(n * HW, (n + 1) * HW)
                    for k in range(kt):
                        nc.tensor.matmul(
                            ps[:, nsl], xT[:, k, :], w_sb[:, k, nsl],
                            start=(k == 0), stop=(k == kt - 1 and bias_row is None),
                        )
                    if bias_row is not None:
                        nc.tensor.matmul(
                            ps[:, nsl], ones_sb[:], bias_row[:, nsl],
                            start=False, stop=True,
                        )
                return ps

            # ================= layer section =================
            with (
                tc.tile_pool(name="wpool", bufs=1) as wpool,
                tc.tile_pool(name="lconsts", bufs=1) as lconsts,
                tc.tile_pool(name="work", bufs=1) as work,
                tc.tile_pool(name="carryp", bufs=1) as carryp,
            ):
                carries = [carryp.tile([1, d], f32r, tag=f"carry{s}", name=f"carry{s}") for s in range(n_seq)]
                for li in range(n_layers):
                    wx_sb = wpool.tile([P, kt, d], f32r, tag="wx")
                    nc.sync.dma_start(out=wx_sb[:], in_=WX.ap()[li].rearrange("k p n -> p k n"))
                    wv_sb = wpool.tile([P, kt, d], f32r, tag="wv")
                    nc.sync.dma_start(out=wv_sb[:], in_=WV.ap()[li].rearrange("k p n -> p k n"))
                    wh_sb = wpool.tile([P, kt, d], f32r, tag="wh")
                    nc.sync.dma_start(out=wh_sb[:], in_=WH.ap()[li].rearrange("k p n -> p k n"))
                    lb_sb = [lconsts.tile([1, d], f32r, tag=f"lb{j}", name=f"lb{j}") for j in range(3)]
                    for j in range(3):
                        nc.sync.dma_start(out=lb_sb[j][:], in_=LB.ap()[li, j : j + 1])
                    rot_sb = lconsts.tile([P, d], f32, tag="rot")
                    nc.sync.dma_start(out=rot_sb[:], in_=ROT.ap()[li])
                    phi_sb = lconsts.tile([1, d], f32, tag="phi")
                    nc.sync.dma_start(out=phi_sb[:], in_=PHI.ap()[li])
                    cosv, sinv = rot_sb[:, 0::2], rot_sb[:, 1::2]

                    for c in range(n_tiles):
                        for s in range(n_seq):
                            a_t = apool.tile([P, d], f32, tag="a")
                            if li == 0:
                                oh_sb = work.tile([P, P], f32r, tag="ohsb", bufs=1)
                                nc.sync.dma_start(out=oh_sb[:], in_=oh.ap()[s, c])
                                aps = uvps.tile([P, d], f32, tag="uv")
                                for n in range(nh):
                                    nsl = slice(n * HW, (n + 1) * HW)
                                    nc.tensor.matmul(
                                        aps[:, nsl], oh_sb[:], emb_sb[:, nsl],
                                        start=True, stop=True,
                                    )
                                    nc.any.tensor_copy(out=a_t[:, nsl], in_=aps[:, nsl])
                            else:
                                nc.sync.dma_start(out=a_t[:], in_=a_dram.ap()[s, c])

                            xh = ln_xhat(work, a_t, "")
                            xT = transpose_in(work, xh, "")

                            u_ps = mm_full(xT, wx_sb, lb_sb[0][:])
                            w_t = work.tile([P, d], f32r, tag="w", bufs=1)
                            t1 = work.tile([P, nb], f32, tag="rt1")
                            t2 = work.tile([P, nb], f32, tag="rt2")
                            ux, uy = u_ps[:, 0::2], u_ps[:, 1::2]
                            nc.vector.tensor_mul(out=t1[:], in0=cosv, in1=ux)
                            nc.vector.tensor_mul(out=t2[:], in0=sinv, in1=uy)
                            nc.vector.tensor_tensor(out=w_t[:, 0::2], in0=t1[:], in1=t2[:], op=add)
                            nc.vector.tensor_mul(out=t1[:], in0=cosv, in1=uy)
                            nc.vector.tensor_mul(out=t2[:], in0=sinv, in1=ux)
                            nc.vector.tensor_tensor(out=w_t[:, 1::2], in0=t1[:], in1=t2[:], op=sub)

                            v_ps = mm_full(xT, wv_sb, lb_sb[1][:])
                            v_t = work.tile([P, d], f32, tag="v", bufs=2)
                            for n in range(nh):
                                nsl = slice(n * HW, (n + 1) * HW)
                                nc.any.tensor_copy(out=v_t[:, nsl], in_=v_ps[:, nsl])

                            p_ps = pps.tile([P, d], f32, tag="p")
                            for n in range(nh):
                                nsl = slice(n * HW, (n + 1) * HW)
                                nc.tensor.matmul(
                                    p_ps[:, nsl], triu_sb[:], w_t[:, nsl],
                                    start=True, stop=(c == 0),
                                )
                                if c > 0:
                                    nc.tensor.matmul(
                                        p_ps[:, nsl], ones_sb[:], carries[s][:, nsl],
                                        start=False, stop=True,
                                    )

                            if c < n_tiles - 1:
                                for n in range(nh):
                                    nsl = slice(n * HW, (n + 1) * HW)
                                    hb = HW // 2
                                    cp = cps.tile([1, HW], f32, tag="cp")
                                    nc.tensor.matmul(
                                        cp[:], triu_sb[:, 127:128], w_t[:, nsl],
                                        start=True, stop=(c == 0),
                                    )
                                    if c > 0:
                                        nc.tensor.matmul(
                                            cp[:], ones_sb[:, 0:1], carries[s][:, nsl],
                                            start=False, stop=True,
                                        )
                                    cpx, cpy = cp[:, 0::2], cp[:, 1::2]
                                    phn = phi_sb[:, nsl]
                                    pc, psn = phn[:, 0::2], phn[:, 1::2]
                                    cout = carries[s][:, nsl]
                                    c1 = stats.tile([1, hb], f32, tag="c1", bufs=2)
                                    c2 = stats.tile([1, hb], f32, tag="c2", bufs=2)
                                    nc.vector.tensor_mul(out=c1[:], in0=pc, in1=cpx)
                                    nc.vector.tensor_mul(out=c2[:], in0=psn, in1=cpy)
                                    nc.vector.tensor_tensor(out=cout[:, 0::2], in0=c1[:], in1=c2[:], op=sub)
                                    nc.vector.tensor_mul(out=c1[:], in0=psn, in1=cpx)
                                    nc.vector.tensor_mul(out=c2[:], in0=pc, in1=cpy)
                                    nc.vector.tensor_tensor(out=cout[:, 1::2], in0=c1[:], in1=c2[:], op=add)

                            hx = work.tile([P, nb], f32, tag="hx")
                            hy = work.tile([P, nb], f32, tag="hy")
                            px, py = p_ps[:, 0::2], p_ps[:, 1::2]
                            nc.vector.tensor_mul(out=t1[:], in0=cosv, in1=px)
                            nc.vector.tensor_mul(out=t2[:], in0=sinv, in1=py)
                            nc.vector.tensor_tensor(out=hx[:], in0=t1[:], in1=t2[:], op=sub)
                            nc.vector.tensor_mul(out=t1[:], in0=sinv, in1=px)
                            nc.vector.tensor_mul(out=t2[:], in0=cosv, in1=py)
                            nc.vector.tensor_tensor(out=hy[:], in0=t1[:], in1=t2[:], op=add)
                            hv = work.tile([P, d], f32r, tag="hv", bufs=1)
                            nc.vector.tensor_mul(out=hv[:, 0::2], in0=hx[:], in1=v_t[:, 0::2])
                            nc.vector.tensor_mul(out=hv[:, 1::2], in0=hy[:], in1=v_t[:, 1::2])

                            hvT = transpose_in(work, hv, "h")
                            bo_ps = mm_full(hvT, wh_sb, lb_sb[2][:])
                            if li < n_layers - 1:
                                for n in range(nh):
                                    nsl = slice(n * HW, (n + 1) * HW)
                                    nc.vector.tensor_tensor(
                                        out=a_t[:, nsl], in0=a_t[:, nsl],
                                        in1=bo_ps[:, nsl], op=add,
                                    )
                            else:
                                for n in range(nh):
                                    nsl = slice(n * HW, (n + 1) * HW)
                                    nc.any.tensor_copy(out=a_t[:, nsl], in_=bo_ps[:, nsl])
                            nc.sync.dma_start(out=a_dram.ap()[s, c], in_=a_t[:])

            # ================= head section =================
            with (
                tc.tile_pool(name="hweights", bufs=1) as hweights,
                tc.tile_pool(name="hwork", bufs=1) as hwork,
            ):
                wo1_sb = hweights.tile([P, kt, d], f32r, tag="wo1")
                nc.sync.dma_start(out=wo1_sb[:], in_=WO1.ap().rearrange("k p n -> p k n"))
                bo1_sb = hweights.tile([1, d], f32r, tag="bo1")
                nc.sync.dma_start(out=bo1_sb[:], in_=BO1.ap())
                wo2_sb = hweights.tile([P, kt, M], f32r, tag="wo2")
                nc.sync.dma_start(out=wo2_sb[:], in_=WO2.ap().rearrange("k p n -> p k n"))
                bo2_sb = hweights.tile([1, M], f32r, tag="bo2")
                nc.sync.dma_start(out=bo2_sb[:], in_=BO2.ap())

                for c in range(n_tiles):
                    for s in range(n_seq):
                        a_t = apool.tile([P, d], f32, tag="a")
                        nc.sync.dma_start(out=a_t[:], in_=a_dram.ap()[s, c])
                        xh = ln_xhat(hwork, a_t, "H")
                        xT = transpose_in(hwork, xh, "H")
                        r_ps = mm_full(xT, wo1_sb, bo1_sb)
                        r_t = hwork.tile([P, d], f32r, tag="r", bufs=2)
                        for n in range(nh):
                            nsl = slice(n * HW, (n + 1) * HW)
                            nc.scalar.activation(
                                out=r_t[:, nsl], in_=r_ps[:, nsl],
                                func=mybir.ActivationFunctionType.Relu,
                            )
                        rT = transpose_in(hwork, r_t, "R")
                        lg_ps = pps.tile([P, M], f32, tag="p")
                        for k in range(kt):
                            nc.tensor.matmul(
                                lg_ps[:], rT[:, k, :], wo2_sb[:, k, :],
                                start=(k == 0), stop=False,
                            )
                        nc.tensor.matmul(
                            lg_ps[:], ones_sb[:], bo2_sb[:], start=False, stop=True
                        )
                        lg = hwork.tile([P, M], f32, tag="lg", bufs=2)
                        nc.any.tensor_copy(out=lg[:], in_=lg_ps[:])
                        nc.sync.dma_start(out=OUT.ap()[s, c], in_=lg[:])
    return nc


def prep_inputs(input_x, embedding, theta, Wx, bx, Wv, bv, Wh, bh,
                ln_g, ln_b, lnf_g, lnf_b, Wo1, bo1, Wo2, bo2,
                n_layers=L, n_seq=BPC, n_tiles=TILES, d=D):
    kt = d // P
    f = np.float32
    WXs = np.empty((n_layers, kt, P, d), f)
    WVs = np.empty((n_layers, kt, P, d), f)
    WHs = np.empty((n_layers, kt, P, d), f)
    LBs = np.empty((n_layers, 3, d), f)
    ROTs = np.empty((n_layers, P, d), f)
    PHIs = np.empty((n_layers, 1, d), f)
    for i in range(n_layers):
        g = ln_g[i].astype(np.float64)
        b = ln_b[i].astype(np.float64)
        WXs[i] = (g[:, None] * np.asarray(Wx[i], np.float64)).astype(f).reshape(kt, P, d)
        WVs[i] = (g[:, None] * np.asarray(Wv[i], np.float64)).astype(f).reshape(kt, P, d)
        WHs[i] = np.asarray(Wh[i], f).reshape(kt, P, d)
        LBs[i, 0] = (b @ np.asarray(Wx[i], np.float64) + np.asarray(bx[i], np.float64)).astype(f)
        LBs[i, 1] = (b @ np.asarray(Wv[i], np.float64) + np.asarray(bv[i], np.float64)).astype(f)
        LBs[i, 2] = np.asarray(bh[i], f)
        ROTs[i], PHIs[i] = _host_tables(np.asarray(theta[i]), d)
    gf = np.asarray(lnf_g, np.float64)
    bf = np.asarray(lnf_b, np.float64)
    WO1s = (gf[:, None] * np.asarray(Wo1, np.float64)).astype(f).reshape(kt, P, d)
    BO1s = (bf @ np.asarray(Wo1, np.float64) + np.asarray(bo1, np.float64)).astype(f)[None, :]
    WO2s = np.asarray(Wo2, f).reshape(kt, P, M)
    BO2s = np.asarray(bo2, f)[None, :]
    shared = {
        "emb": np.asarray(embedding, f), "WX": WXs, "WV": WVs, "WH": WHs,
        "LB": LBs, "ROT": ROTs, "PHI": PHIs,
        "TRIU": np.triu(np.ones((P, P), f)), "ONES1": np.ones((1, P), f),
        "IDENT": np.eye(P, dtype=f),
        "WO1": WO1s, "BO1": BO1s, "WO2": WO2s, "BO2": BO2s,
    }
    per_core = []
    x_all = np.asarray(input_x)
    n_cores = x_all.shape[0] // n_seq
    for ci in range(n_cores):
        xb = x_all[ci * n_seq : (ci + 1) * n_seq]
        ohc = np.zeros((n_seq, n_tiles, P, P), f)
        for si in range(n_seq):
            toks = xb[si].astype(np.int64).reshape(n_tiles, P)
            for ti in range(n_tiles):
                ohc[si, ti, toks[ti], np.arange(P)] = 1.0  # [vocab, tok]
        per_core.append({**shared, "oh": ohc})
    return per_core


_NC_CACHE = {}


def _get_nc():
    if "nc" not in _NC_CACHE:
        nc = bacc.Bacc("TRN2", target_bir_lowering=False, debug=False,
                       num_devices=NCORES)
        build(nc)
        nc.compile()
        _NC_CACHE["nc"] = nc
    return _NC_CACHE["nc"]


def kernel(**inputs):
    from concourse.bass_utils import run_bass_kernel_spmd

    nc = _get_nc()
    in_maps = prep_inputs(**inputs)
    res = run_bass_kernel_spmd(nc, in_maps, core_ids=list(range(NCORES)))
    outs = []
    for ci in range(NCORES):
        o = res.results[ci]["out"]  # [n_seq, n_tiles, P, M]
        outs.append(np.asarray(o).reshape(BPC, S, M))
    return np.concatenate(outs, axis=0).astype(np.float32)


# revision 17
# speedup vs baseline: 16.9576x; 16.9576x over previous
"""Trainium2 Bass kernel for nn_CycleMultiBlock (self-contained).

Model: a = emb[x]; L x [a (+)= ((scan(LN(a)@Wx) * (LN(a)@Wv)) @ Wh)]; LN; MLP head.
The linear RNN h_t = R(theta) h_{t-1} + u_t (2x2 rotation blocks, theta constant
over time) is factored as h_t = e^{i t th} * cumsum_k(e^{-i k th} u_k), computed
per 128-token tile with a triangular-ones matmul; the cross-tile carry is
injected into PSUM with a K=1 ones-row matmul.

Sharding: data-parallel over batch; 2 sequences per core, 8 cores.
Activations token-major [128 tok, D]; lhsT tiles via PE transposes.
Matmuls in float32r (full-rate fp32, ~1e-4 input rounding).
"""
import sys

sys.path.insert(0, "/opt/trn_rl_repo")

import numpy as np

import concourse.bacc as bacc
import concourse.mybir as mybir
import concourse.tile as tile

B, S, M, D, L = 16, 2048, 128, 1024, 4
P = 128
NCORES = 8
BPC = B // NCORES          # sequences per core
TILES = S // P             # 128-token tiles per sequence
EPS = 1e-5

f32 = mybir.dt.float32
f32r = mybir.dt.float32r

# engine assignment for SBUF-only elementwise ops ("vector" or "gpsimd")
import os as _os
OPTS = {"xhat": "vector", "wcomb": "gpsimd", "hv": "gpsimd", "ccomb": "gpsimd"}
if _os.environ.get("KOPTS"):
    for _kv in _os.environ["KOPTS"].split(","):
        _k, _v = _kv.split("=")
        OPTS[_k] = _v


def _host_tables(theta_l, d):
    th = theta_l.astype(np.float64)  # [d//2]
    k = np.arange(P, dtype=np.float64)[:, None]
    ang = k * th[None, :]
    rot = np.empty((P, d), np.float32)
    rot[:, 0::2] = np.cos(ang).astype(np.float32)
    rot[:, 1::2] = np.sin(ang).astype(np.float32)
    angp = P * th
    phi = np.empty((1, d), np.float32)
    phi[0, 0::2] = np.cos(angp).astype(np.float32)
    phi[0, 1::2] = np.sin(angp).astype(np.float32)
    return rot, phi


def build(nc, n_layers=L, n_seq=BPC, n_tiles=TILES, d=D):
    kt = d // P
    nb = d // 2
    nh = max(1, d // 512)              # 512-wide psum halves
    HW = min(512, d)
    oh = nc.dram_tensor("oh", (n_seq, n_tiles, P, P), f32r, kind="ExternalInput")
    emb = nc.dram_tensor("emb", (P, d), f32r, kind="ExternalInput")
    WX = nc.dram_tensor("WX", (n_layers, kt, P, d), f32r, kind="ExternalInput")
    WV = nc.dram_tensor("WV", (n_layers, kt, P, d), f32r, kind="ExternalInput")
    WH = nc.dram_tensor("WH", (n_layers, kt, P, d), f32r, kind="ExternalInput")
    LB = nc.dram_tensor("LB", (n_layers, 3, d), f32r, kind="ExternalInput")
    ROT = nc.dram_tensor("ROT", (n_layers, P, d), f32, kind="ExternalInput")
    PHI = nc.dram_tensor("PHI", (n_layers, 1, d), f32, kind="ExternalInput")
    TRIU = nc.dram_tensor("TRIU", (P, P), f32r, kind="ExternalInput")
    ONES1 = nc.dram_tensor("ONES1", (1, P), f32r, kind="ExternalInput")
    IDENT = nc.dram_tensor("IDENT", (P, P), f32r, kind="ExternalInput")
    WO1 = nc.dram_tensor("WO1", (kt, P, d), f32r, kind="ExternalInput")
    BO1 = nc.dram_tensor("BO1", (1, d), f32r, kind="ExternalInput")
    WO2 = nc.dram_tensor("WO2", (kt, P, M), f32r, kind="ExternalInput")
    BO2 = nc.dram_tensor("BO2", (1, M), f32r, kind="ExternalInput")
    a_dram = nc.dram_tensor("a_scratch", (n_seq, n_tiles, P, d), f32)
    OUT = nc.dram_tensor("out", (n_seq, n_tiles, P, M), f32, kind="ExternalOutput")

    sub, mult, add = (
        mybir.AluOpType.subtract,
        mybir.AluOpType.mult,
        mybir.AluOpType.add,
    )

    with tile.TileContext(nc) as tc:
        with (
            tc.tile_pool(name="consts", bufs=1) as consts,
            tc.tile_pool(name="apool", bufs=3) as apool,
            tc.tile_pool(name="stats", bufs=4) as stats,
            tc.tile_pool(name="uvps", bufs=3, space="PSUM") as uvps,
            tc.tile_pool(name="pps", bufs=2, space="PSUM") as pps,
            tc.tile_pool(name="trps", bufs=2, space="PSUM") as trps,
            tc.tile_pool(name="cps", bufs=1, space="PSUM") as cps,
        ):
            triu_sb = consts.tile([P, P], f32r)
            nc.sync.dma_start(out=triu_sb[:], in_=TRIU.ap())
            ones_sb = consts.tile([1, P], f32r)
            nc.sync.dma_start(out=ones_sb[:], in_=ONES1.ap())
            ident_sb = consts.tile([P, P], f32r)
            nc.sync.dma_start(out=ident_sb[:], in_=IDENT.ap())
            emb_sb = consts.tile([P, d], f32r)
            nc.sync.dma_start(out=emb_sb[:], in_=emb.ap())
            eps_sb = consts.tile([P, 1], f32)
            nc.vector.memset(eps_sb[:], EPS)

            def ln_xhat(pool, a_t, tag):
                st = stats.tile([P, 2, nc.vector.BN_STATS_DIM], f32, tag=f"st{tag}")
                half = d // 2
                for g in range(2):
                    nc.vector.bn_stats(
                        out=st[:, g, :], in_=a_t[:, g * half : (g + 1) * half]
                    )
                mv = stats.tile([P, nc.vector.BN_AGGR_DIM], f32, tag=f"mv{tag}")
                nc.vector.bn_aggr(out=mv[:], in_=st[:])
                rs = stats.tile([P, 1], f32, tag=f"rs{tag}")
                nc.scalar.activation(
                    out=rs[:], in_=mv[:, 1:2],
                    func=mybir.ActivationFunctionType.Sqrt,
                    bias=eps_sb[:], scale=1.0,
                )
                nc.vector.reciprocal(out=rs[:], in_=rs[:])
                xh = pool.tile([P, d], f32r, tag=f"xh{tag}", bufs=2)
                getattr(nc, OPTS["xhat"]).tensor_scalar(
                    out=xh[:], in0=a_t[:], scalar1=mv[:, 0:1], scalar2=rs[:],
                    op0=sub, op1=mult,
                )
                return xh

            def transpose_in(pool, x_t, tag):
                xT = pool.tile([P, kt, P], f32r, tag=f"xT{tag}", bufs=2)
                for k in range(kt):
                    tp = trps.tile([P, P], x_t.dtype, tag="trp")
                    nc.tensor.transpose(tp[:], x_t[:, k * P : (k + 1) * P], ident_sb[:])
                    nc.any.tensor_copy(out=xT[:, k, :], in_=tp[:])
                return xT

            def mm_half(xT, w_sb, bias_row, n):
                """psum [P, HW] = (xT.T @ W + bias)[:, half n]."""
                ps = uvps.tile([P, HW], f32, tag="uv")
                nsl = slice(n * HW, (n + 1) * HW)
                for k in range(kt):
                    nc.tensor.matmul(
                        ps[:], xT[:, k, :], w_sb[:, k, nsl],
                        start=(k == 0), stop=(k == kt - 1 and bias_row is None),
                    )
                if bias_row is not None:
                    nc.tensor.matmul(
                        ps[:], ones_sb[:], bias_row[:, nsl],
                        start=False, stop=True,
                    )
                return ps

            # ================= layer section =================
            with (
                tc.tile_pool(name="wpool", bufs=1) as wpool,
                tc.tile_pool(name="lconsts", bufs=1) as lconsts,
                tc.tile_pool(name="work", bufs=1) as work,
                tc.tile_pool(name="carryp", bufs=1) as carryp,
            ):
                carries = [carryp.tile([1, d], f32r, tag=f"carry{s}", name=f"carry{s}") for s in range(n_seq)]
                for li in range(n_layers):
                    wx_sb = wpool.tile([P, kt, d], f32r, tag="wx")
                    nc.sync.dma_start(out=wx_sb[:], in_=WX.ap()[li].rearrange("k p n -> p k n"))
                    wv_sb = wpool.tile([P, kt, d], f32r, tag="wv")
                    nc.sync.dma_start(out=wv_sb[:], in_=WV.ap()[li].rearrange("k p n -> p k n"))
                    wh_sb = wpool.tile([P, kt, d], f32r, tag="wh")
                    nc.sync.dma_start(out=wh_sb[:], in_=WH.ap()[li].rearrange("k p n -> p k n"))
                    lb_sb = [lconsts.tile([1, d], f32r, tag=f"lb{j}", name=f"lb{j}") for j in range(3)]
                    for j in range(3):
                        nc.sync.dma_start(out=lb_sb[j][:], in_=LB.ap()[li, j : j + 1])
                    rot_sb = lconsts.tile([P, d], f32, tag="rot")
                    nc.sync.dma_start(out=rot_sb[:], in_=ROT.ap()[li])
                    phi_sb = lconsts.tile([1, d], f32, tag="phi")
                    nc.sync.dma_start(out=phi_sb[:], in_=PHI.ap()[li])
                    cosv, sinv = rot_sb[:, 0::2], rot_sb[:, 1::2]

                    for c in range(n_tiles):
                        for s in range(n_seq):
                            a_t = apool.tile([P, d], f32, tag="a")
                            if li == 0:
                                oh_sb = work.tile([P, P], f32r, tag="ohsb", bufs=1)
                                nc.sync.dma_start(out=oh_sb[:], in_=oh.ap()[s, c])
                                for n in range(nh):
                                    nsl = slice(n * HW, (n + 1) * HW)
                                    aps = uvps.tile([P, HW], f32, tag="uv")
                                    nc.tensor.matmul(
                                        aps[:], oh_sb[:], emb_sb[:, nsl],
                                        start=True, stop=True,
                                    )
                                    nc.any.tensor_copy(out=a_t[:, nsl], in_=aps[:])
                            else:
                                nc.sync.dma_start(out=a_t[:], in_=a_dram.ap()[s, c])

                            xh = ln_xhat(work, a_t, "")
                            xT = transpose_in(work, xh, "")

                            hb = HW // 2
                            w_t = work.tile([P, d], f32r, tag="w", bufs=2)
                            v_t = work.tile([P, d], f32, tag="v", bufs=2)
                            hv = work.tile([P, d], f32r, tag="hv", bufs=2)
                            for n in range(nh):
                                nsl = slice(n * HW, (n + 1) * HW)
                                cosn, sinn = cosv[:, n * hb : (n + 1) * hb], sinv[:, n * hb : (n + 1) * hb]
                                wsl = w_t[:, nsl]
                                u_ps = mm_half(xT, wx_sb, lb_sb[0][:], n)
                                t1 = work.tile([P, hb], f32, tag="rt1", bufs=2)
                                t2 = work.tile([P, hb], f32, tag="rt2", bufs=2)
                                t3 = work.tile([P, hb], f32, tag="rt3", bufs=2)
                                t4 = work.tile([P, hb], f32, tag="rt4", bufs=2)
                                ux, uy = u_ps[:, 0::2], u_ps[:, 1::2]
                                nc.vector.tensor_mul(out=t1[:], in0=cosn, in1=ux)
                                nc.vector.tensor_mul(out=t2[:], in0=sinn, in1=uy)
                                nc.vector.tensor_mul(out=t3[:], in0=cosn, in1=uy)
                                nc.vector.tensor_mul(out=t4[:], in0=sinn, in1=ux)
                                getattr(nc, OPTS["wcomb"]).tensor_tensor(out=wsl[:, 0::2], in0=t1[:], in1=t2[:], op=add)
                                getattr(nc, OPTS["wcomb"]).tensor_tensor(out=wsl[:, 1::2], in0=t3[:], in1=t4[:], op=sub)

                                v_ps = mm_half(xT, wv_sb, lb_sb[1][:], n)
                                nc.any.tensor_copy(out=v_t[:, nsl], in_=v_ps[:])

                                p_ps = pps.tile([P, HW], f32, tag="p")
                                nc.tensor.matmul(
                                    p_ps[:], triu_sb[:], wsl,
                                    start=True, stop=(c == 0),
                                )
                                if c > 0:
                                    nc.tensor.matmul(
                                        p_ps[:], ones_sb[:], carries[s][:, nsl],
                                        start=False, stop=True,
                                    )

                                if c < n_tiles - 1:
                                    cp = cps.tile([1, HW], f32, tag="cp")
                                    nc.tensor.matmul(
                                        cp[:], triu_sb[:, 127:128], wsl,
                                        start=True, stop=(c == 0),
                                    )
                                    if c > 0:
                                        nc.tensor.matmul(
                                            cp[:], ones_sb[:, 0:1], carries[s][:, nsl],
                                            start=False, stop=True,
                                        )
                                    cpx, cpy = cp[:, 0::2], cp[:, 1::2]
                                    phn = phi_sb[:, nsl]
                                    pc, psn = phn[:, 0::2], phn[:, 1::2]
                                    cout = carries[s][:, nsl]
                                    c1 = stats.tile([1, hb], f32, tag="c1", bufs=2)
                                    c2 = stats.tile([1, hb], f32, tag="c2", bufs=2)
                                    nc.vector.tensor_mul(out=c1[:], in0=pc, in1=cpx)
                                    nc.vector.tensor_mul(out=c2[:], in0=psn, in1=cpy)
                                    getattr(nc, OPTS["ccomb"]).tensor_tensor(out=cout[:, 0::2], in0=c1[:], in1=c2[:], op=sub)
                                    nc.vector.tensor_mul(out=c1[:], in0=psn, in1=cpx)
                                    nc.vector.tensor_mul(out=c2[:], in0=pc, in1=cpy)
                                    getattr(nc, OPTS["ccomb"]).tensor_tensor(out=cout[:, 1::2], in0=c1[:], in1=c2[:], op=add)

                                hx = work.tile([P, hb], f32, tag="hx", bufs=2)
                                hy = work.tile([P, hb], f32, tag="hy", bufs=2)
                                t5 = work.tile([P, hb], f32, tag="rt1", bufs=2, name="t5")
                                t6 = work.tile([P, hb], f32, tag="rt2", bufs=2, name="t6")
                                px, py = p_ps[:, 0::2], p_ps[:, 1::2]
                                nc.vector.tensor_mul(out=hx[:], in0=cosn, in1=px)
                                nc.vector.tensor_mul(out=t5[:], in0=sinn, in1=py)
                                nc.vector.tensor_mul(out=hy[:], in0=sinn, in1=px)
                                nc.vector.tensor_mul(out=t6[:], in0=cosn, in1=py)
                                nc.vector.tensor_tensor(out=hx[:], in0=hx[:], in1=t5[:], op=sub)
                                nc.vector.tensor_tensor(out=hy[:], in0=hy[:], in1=t6[:], op=add)
                                getattr(nc, OPTS["hv"]).tensor_mul(out=hv[:, nsl][:, 0::2], in0=hx[:], in1=v_t[:, nsl][:, 0::2])
                                getattr(nc, OPTS["hv"]).tensor_mul(out=hv[:, nsl][:, 1::2], in0=hy[:], in1=v_t[:, nsl][:, 1::2])

                            hvT = transpose_in(work, hv, "h")
                            for n in range(nh):
                                nsl = slice(n * HW, (n + 1) * HW)
                                bo_ps = mm_half(hvT, wh_sb, lb_sb[2][:], n)
                                if li < n_layers - 1:
                                    nc.vector.tensor_tensor(
                                        out=a_t[:, nsl], in0=a_t[:, nsl],
                                        in1=bo_ps[:], op=add,
                                    )
                                else:
                                    nc.any.tensor_copy(out=a_t[:, nsl], in_=bo_ps[:])
                            nc.sync.dma_start(out=a_dram.ap()[s, c], in_=a_t[:])

                # ================= head (same pools: overlaps layer 3) =================
                wo1_sb = wpool.tile([P, kt, d], f32r, tag="wx", name="wo1_sb")
                nc.sync.dma_start(out=wo1_sb[:], in_=WO1.ap().rearrange("k p n -> p k n"))
                bo1_sb = lconsts.tile([1, d], f32r, tag="lb0", name="bo1_sb")
                nc.sync.dma_start(out=bo1_sb[:], in_=BO1.ap())
                wo2_sb = wpool.tile([P, kt, M], f32r, tag="wv", name="wo2_sb")
                nc.sync.dma_start(out=wo2_sb[:], in_=WO2.ap().rearrange("k p n -> p k n"))
                bo2_sb = lconsts.tile([1, M], f32r, tag="lb1", name="bo2_sb")
                nc.sync.dma_start(out=bo2_sb[:], in_=BO2.ap())

                for c in range(n_tiles):
                    for s in range(n_seq):
                        a_t = apool.tile([P, d], f32, tag="a")
                        nc.sync.dma_start(out=a_t[:], in_=a_dram.ap()[s, c])
                        xh = ln_xhat(work, a_t, "")
                        xT = transpose_in(work, xh, "")
                        r_t = work.tile([P, d], f32r, tag="w", name="r_t", bufs=2)
                        for n in range(nh):
                            nsl = slice(n * HW, (n + 1) * HW)
                            r_ps = mm_half(xT, wo1_sb, bo1_sb, n)
                            nc.scalar.activation(
                                out=r_t[:, nsl], in_=r_ps[:],
                                func=mybir.ActivationFunctionType.Relu,
                            )
                        rT = transpose_in(work, r_t, "h")
                        lg_ps = pps.tile([P, M], f32, tag="p", name="lg_ps")
                        for k in range(kt):
                            nc.tensor.matmul(
                                lg_ps[:], rT[:, k, :], wo2_sb[:, k, :],
                                start=(k == 0), stop=False,
                            )
                        nc.tensor.matmul(
                            lg_ps[:], ones_sb[:], bo2_sb[:], start=False, stop=True
                        )
                        lg = work.tile([P, M], f32, tag="ohsb", name="lg")
                        nc.any.tensor_copy(out=lg[:], in_=lg_ps[:])
                        nc.sync.dma_start(out=OUT.ap()[s, c], in_=lg[:])
    return nc


def prep_inputs(input_x, embedding, theta, Wx, bx, Wv, bv, Wh, bh,
                ln_g, ln_b, lnf_g, lnf_b, Wo1, bo1, Wo2, bo2,
                n_layers=L, n_seq=BPC, n_tiles=TILES, d=D):
    kt = d // P
    f = np.float32
    WXs = np.empty((n_layers, kt, P, d), f)
    WVs = np.empty((n_layers, kt, P, d), f)
    WHs = np.empty((n_layers, kt, P, d), f)
    LBs = np.empty((n_layers, 3, d), f)
    ROTs = np.empty((n_layers, P, d), f)
    PHIs = np.empty((n_layers, 1, d), f)
    for i in range(n_layers):
        g = ln_g[i].astype(np.float64)
        b = ln_b[i].astype(np.float64)
        WXs[i] = (g[:, None] * np.asarray(Wx[i], np.float64)).astype(f).reshape(kt, P, d)
        WVs[i] = (g[:, None] * np.asarray(Wv[i], np.float64)).astype(f).reshape(kt, P, d)
        WHs[i] = np.asarray(Wh[i], f).reshape(kt, P, d)
        LBs[i, 0] = (b @ np.asarray(Wx[i], np.float64) + np.asarray(bx[i], np.float64)).astype(f)
        LBs[i, 1] = (b @ np.asarray(Wv[i], np.float64) + np.asarray(bv[i], np.float64)).astype(f)
        LBs[i, 2] = np.asarray(bh[i], f)
        ROTs[i], PHIs[i] = _host_tables(np.asarray(theta[i]), d)
    gf = np.asarray(lnf_g, np.float64)
    bf = np.asarray(lnf_b, np.float64)
    WO1s = (gf[:, None] * np.asarray(Wo1, np.float64)).astype(f).reshape(kt, P, d)
    BO1s = (bf @ np.asarray(Wo1, np.float64) + np.asarray(bo1, np.float64)).astype(f)[None, :]
    WO2s = np.asarray(Wo2, f).reshape(kt, P, M)
    BO2s = np.asarray(bo2, f)[None, :]
    shared = {
        "emb": np.asarray(embedding, f), "WX": WXs, "WV": WVs, "WH": WHs,
        "LB": LBs, "ROT": ROTs, "PHI": PHIs,
        "TRIU": np.triu(np.ones((P, P), f)), "ONES1": np.ones((1, P), f),
        "IDENT": np.eye(P, dtype=f),
        "WO1": WO1s, "BO1": BO1s, "WO2": WO2s, "BO2": BO2s,
    }
    per_core = []
    x_all = np.asarray(input_x)
    n_cores = x_all.shape[0] // n_seq
    for ci in range(n_cores):
        xb = x_all[ci * n_seq : (ci + 1) * n_seq]
        ohc = np.zeros((n_seq, n_tiles, P, P), f)
        for si in range(n_seq):
            toks = xb[si].astype(np.int64).reshape(n_tiles, P)
            for ti in range(n_tiles):
                ohc[si, ti, toks[ti], np.arange(P)] = 1.0  # [vocab, tok]
        per_core.append({**shared, "oh": ohc})
    return per_core


_NC_CACHE = {}


def _get_nc():
    if "nc" not in _NC_CACHE:
        nc = bacc.Bacc("TRN2", target_bir_lowering=False, debug=False,
                       num_devices=NCORES)
        build(nc)
        nc.compile()
        _NC_CACHE["nc"] = nc
    return _NC_CACHE["nc"]


def kernel(**inputs):
    from concourse.bass_utils import run_bass_kernel_spmd

    nc = _get_nc()
    in_maps = prep_inputs(**inputs)
    res = run_bass_kernel_spmd(nc, in_maps, core_ids=list(range(NCORES)))
    outs = []
    for ci in range(NCORES):
        o = res.results[ci]["out"]  # [n_seq, n_tiles, P, M]
        outs.append(np.asarray(o).reshape(BPC, S, M))
    return np.concatenate(outs, axis=0).astype(np.float32)
